# revision 22
# baseline (speedup 1.0000x reference)
"""Otsu-threshold binarization (histogram_binning) as a Bass/Tile kernel on 8 TRN2 cores.

Pipeline per core (data-parallel over batch, shard = 4 of 32 batches):
  1. Stream input, fuse RGB->gray (fp32, DVE); per-chunk min/max on Pool.
  2. Cross-partition reduce + AllReduce(max) of [-vmin, vmax].
  3. Quantize gray to q in [0,256]; split coarse c=min(q>>4,15), fine f=q&15.
     Build 16 ReLU *ramp* planes per side (r_{-1}=x+1, r_a=relu(x-a)),
     plane-major bf16, split across DVE/ACT/Pool engines. The 16x16 joint
     histogram is the double second-difference of the ramp outer-product
     matrix R[b,a] = sum_px rf_b * rc_a, which the PE accumulates as
     block-diagonal 128x128 matmuls (8 value-slots per matmul, fp32 PSUM,
     exact: per-bank partial sums < 2^24).
  4. Diagonal extraction on-chip: mask out[8b+v, 8a+v'] with v==v' (DVE),
     column-sum via constant selector matmul (PE), v+bank reduce in int32,
     then Delta^2 over coarse (DVE) and over fine (tiny constant matmul).
  5. AllReduce(add) of the 256-bin histogram; on-chip Otsu (edges via
     emulated-FMA linspace, fp32 cumsums, argmax via Max8/MaxIndex).
  6. Binarize gray > thresh, replicate to 3 channels, stream out.
"""
import os
import numpy as np

P = 128
NCORES = 8
B, H, WD, C = 32, 128, 2048, 3
BPC = B // NCORES                  # batches per core
FIN = BPC * H * WD * C // P        # 24576 raw values per partition
FPIX = FIN // 3                    # 8192 pixels per partition
NCH = 8                            # streaming chunks
CPIX = FPIX // NCH                 # 1024 pixels per chunk
CIN = CPIX * 3                     # 3072 raw values per chunk
PCH = 512                          # histogram chunk (values per partition)
NPCH = FPIX // PCH                 # 16
G = 8                              # value-slots stacked per matmul
WR, WG, WB = 0.2989, 0.5870, 0.1140

_NC_CACHE = {}


def _build_nc():
    import concourse.mybir as mybir
    import concourse.tile as tile
    from concourse import bacc
    import bass_rust

    dt = mybir.dt
    Alu = mybir.AluOpType
    Ax = mybir.AxisListType
    Act = mybir.ActivationFunctionType
    Red = bass_rust.ReduceOp
    groups = [list(range(NCORES))]

    nc = bacc.Bacc("TRN2", target_bir_lowering=False, debug=False,
                   num_devices=NCORES)
    x_d = nc.dram_tensor("x", [P, FIN], dt.float32, kind="ExternalInput").ap()
    out_d = nc.dram_tensor("out", [P, FIN], dt.float32, kind="ExternalOutput").ap()
    dbg = os.environ.get("KDBG", "") == "1"
    if dbg:
        dbg_d = nc.dram_tensor("dbg", [1, 51200], dt.float32,
                               kind="ExternalOutput").ap()

    with tile.TileContext(nc) as tc:
        with (
            tc.tile_pool(name="gray", bufs=1) as gp,
            tc.tile_pool(name="smol", bufs=1) as sp,
            tc.tile_pool(name="cst", bufs=1) as kp,
            tc.tile_pool(name="psum", bufs=1, space="PSUM") as pp,
            tc.tile_pool(name="dram", bufs=1, space="DRAM") as dp,
        ):
            gray = gp.tile([P, FPIX], dt.float32)
            mns = sp.tile([P, NCH], dt.float32)
            mxs = sp.tile([P, NCH], dt.float32)

            # ---- CC warmup: absorb the collective engine's cold-start ----
            wu_in = dp.tile([1, 2], dt.float32)
            wu_out = dp.tile([1, 2], dt.float32)
            wusb = sp.tile([1, 2], dt.float32)
            nc.gpsimd.memset(wusb[:], 0.0)
            nc.sync.dma_start(wu_in[:], wusb[:])
            nc.gpsimd.collective_compute("AllReduce", Alu.max,
                                         replica_groups=groups,
                                         ins=[wu_in.opt()],
                                         outs=[wu_out.opt()])

            # ---- constants (independent; overlap with input DMA) ----
            # diag mask D[p, n] = 1{(n - p) % 8 == 0}  (v == v')
            ti = kp.tile([P, 128], dt.int32)
            nc.gpsimd.iota(ti[:], pattern=[[1, 128]], base=0,
                           channel_multiplier=-1)
            tand = kp.tile([P, 128], dt.int32)
            nc.vector.tensor_scalar(tand[:], ti[:], 7, None, Alu.bitwise_and)
            dmask = kp.tile([P, 128], dt.float32)
            nc.vector.tensor_scalar(dmask[:], tand[:], 0, None, Alu.is_equal)
            # selector W[p, m] = 1{m == p >> 3}
            prow = kp.tile([P, 16], dt.int32)
            nc.gpsimd.iota(prow[:], pattern=[[0, 16]], base=0,
                           channel_multiplier=1)
            prs = kp.tile([P, 16], dt.int32)
            nc.vector.tensor_scalar(prs[:], prow[:], 3, None,
                                    Alu.logical_shift_right)
            col16 = kp.tile([P, 16], dt.int32)
            nc.gpsimd.iota(col16[:], pattern=[[1, 16]], base=0,
                           channel_multiplier=0)
            wsel = kp.tile([P, 16], dt.float32)
            nc.vector.tensor_tensor(wsel[:], prs[:], col16[:], Alu.is_equal)
            # second-difference matrix Dm[k, m] = +1{k==m} -2{k==m+1} +1{k==m+2}
            t16 = kp.tile([16, 16], dt.int32)
            nc.gpsimd.iota(t16[:], pattern=[[1, 16]], base=0,
                           channel_multiplier=-1)
            dm0 = kp.tile([16, 16], dt.float32)
            nc.vector.tensor_scalar(dm0[:], t16[:], 0, None, Alu.is_equal)
            dm2 = kp.tile([16, 16], dt.float32)
            nc.vector.tensor_scalar(dm2[:], t16[:], -2, None, Alu.is_equal)
            dm1 = kp.tile([16, 16], dt.float32)
            nc.vector.tensor_scalar(dm1[:], t16[:], -1, None, Alu.is_equal)
            dmat = kp.tile([16, 16], dt.float32)
            nc.vector.tensor_tensor(dmat[:], dm0[:], dm2[:], Alu.add)
            nc.vector.scalar_tensor_tensor(dmat[:], dm1[:], -2.0, dmat[:],
                                           Alu.mult, Alu.add)
            # ramp biases for ACT: column i holds 1 - i
            bia = kp.tile([P, 16], dt.int32)
            nc.gpsimd.iota(bia[:], pattern=[[1, 16]], base=0,
                           channel_multiplier=0)
            biasr = kp.tile([P, 16], dt.float32)
            nc.vector.tensor_scalar(biasr[:], bia[:], -1.0, 1.0,
                                    Alu.mult, Alu.add)
            # iota ramp for otsu (t in [0,1])
            io32 = kp.tile([1, 257], dt.int32)
            nc.gpsimd.iota(io32[:], pattern=[[1, 257]], base=0,
                           channel_multiplier=0)
            iof = kp.tile([1, 257], dt.float32)
            nc.vector.tensor_copy(iof[:], io32[:])

            # ---- Phase A: load + grayscale + per-chunk min/max (Pool) ----
            with tc.tile_pool(name="xin", bufs=3) as inp:
                for ch in range(NCH):
                    xt = inp.tile([P, CIN], dt.float32)
                    nc.sync.dma_start(xt[:], x_d[:, ch * CIN:(ch + 1) * CIN])
                    xv = xt[:].rearrange("p (v c) -> p c v", c=3)
                    gs = gray[:, ch * CPIX:(ch + 1) * CPIX]
                    nc.vector.tensor_scalar(gs, xv[:, 0], WR, None, Alu.mult)
                    nc.vector.scalar_tensor_tensor(gs, xv[:, 1], WG, gs,
                                                   Alu.mult, Alu.add)
                    nc.vector.scalar_tensor_tensor(gs, xv[:, 2], WB, gs,
                                                   Alu.mult, Alu.add)
                    nc.vector.tensor_reduce(mns[:, ch:ch + 1], gs, axis=Ax.X,
                                            op=Alu.min)
                    nc.vector.tensor_reduce(mxs[:, ch:ch + 1], gs, axis=Ax.X,
                                            op=Alu.max)

            # ---- Phase B: global min/max ----
            mn = sp.tile([P, 1], dt.float32)
            mx = sp.tile([P, 1], dt.float32)
            nc.vector.tensor_reduce(mn, mns[:], axis=Ax.X, op=Alu.min)
            nc.vector.tensor_reduce(mx, mxs[:], axis=Ax.X, op=Alu.max)
            mm2 = sp.tile([P, 2], dt.float32)   # [-vmin, vmax]
            nc.vector.tensor_scalar(mm2[:, 0:1], mn[:], -1.0, None, Alu.mult)
            nc.vector.tensor_copy(mm2[:, 1:2], mx[:])
            mmr = sp.tile([P, 2], dt.float32)
            nc.gpsimd.partition_all_reduce(mmr[:], mm2[:], channels=P,
                                           reduce_op=Red.max)
            mm_in = dp.tile([1, 2], dt.float32)
            mm_out = dp.tile([1, 2], dt.float32)
            nc.sync.dma_start(mm_in[:], mmr[0:1, :])
            nc.gpsimd.collective_compute("AllReduce", Alu.max,
                                         replica_groups=groups,
                                         ins=[mm_in.opt()],
                                         outs=[mm_out.opt()])
            mmg = sp.tile([1, 2], dt.float32)
            nc.sync.dma_start(mmg[:], mm_out[:])
            mmb = sp.tile([P, 2], dt.float32)  # [:,0] = -vmin, [:,1] = vmax
            nc.gpsimd.partition_broadcast(mmb[:], mmg[:], channels=P)
            negvmin = mmb[:, 0:1]
            vmaxc = mmb[:, 1:2]
            delta = sp.tile([P, 1], dt.float32)
            nc.vector.tensor_tensor(delta[:], vmaxc, negvmin, Alu.add)
            rdel = sp.tile([P, 1], dt.float32)
            nc.vector.reciprocal(rdel[:], delta[:])
            s256 = sp.tile([P, 1], dt.float32)
            nc.vector.tensor_scalar(s256[:], rdel[:], 256.0, None, Alu.mult)
            # HW float->int converts round-to-nearest; pre-subtract half a
            # bin so round(y - 0.5) == trunc(y): A = -vmin - delta/512
            hstep = sp.tile([P, 1], dt.float32)
            nc.vector.tensor_scalar(hstep[:], delta[:], 1.0 / 512.0, None,
                                    Alu.mult)
            nadj = sp.tile([P, 1], dt.float32)
            nc.vector.tensor_tensor(nadj[:], negvmin, hstep[:], Alu.subtract)

            # ---- edges/centers (depend only on mmg; overlap with Phase C) ----
            tt_ = sp.tile([1, 257], dt.float32)
            nc.vector.tensor_scalar(tt_[:], iof[:], 1.0 / 256.0, None,
                                    Alu.mult)
            omt = sp.tile([1, 257], dt.float32)
            nc.vector.tensor_scalar(omt[:], tt_[:], -1.0, 1.0, Alu.mult,
                                    Alu.add)
            vminp = sp.tile([1, 1], dt.float32)
            nc.vector.tensor_scalar(vminp[:], mmg[:, 0:1], -1.0, None,
                                    Alu.mult)
            cpart = sp.tile([1, 257], dt.float32)
            nc.vector.tensor_scalar(cpart[:], omt[:], vminp[:], None,
                                    Alu.mult)
            vx = mmg[:, 1:2]
            # emulated fma(vmax, t, cpart): Veltkamp split + 2Sum
            c1 = sp.tile([1, 1], dt.float32)
            nc.vector.tensor_scalar(c1[:], vx, 4097.0, None, Alu.mult)
            cm = sp.tile([1, 1], dt.float32)
            nc.vector.tensor_tensor(cm[:], c1[:], vx, Alu.subtract)
            ahi = sp.tile([1, 1], dt.float32)
            nc.vector.tensor_tensor(ahi[:], c1[:], cm[:], Alu.subtract)
            alo = sp.tile([1, 1], dt.float32)
            nc.vector.tensor_tensor(alo[:], vx, ahi[:], Alu.subtract)
            pr = sp.tile([1, 257], dt.float32)
            nc.vector.tensor_scalar(pr[:], tt_[:], vx, None, Alu.mult)
            hh = sp.tile([1, 257], dt.float32)
            nc.vector.tensor_scalar(hh[:], tt_[:], ahi[:], None, Alu.mult)
            e0 = sp.tile([1, 257], dt.float32)
            nc.vector.tensor_tensor(e0[:], hh[:], pr[:], Alu.subtract)
            ll = sp.tile([1, 257], dt.float32)
            nc.vector.tensor_scalar(ll[:], tt_[:], alo[:], None, Alu.mult)
            er = sp.tile([1, 257], dt.float32)
            nc.vector.tensor_tensor(er[:], e0[:], ll[:], Alu.add)
            ss = sp.tile([1, 257], dt.float32)
            nc.vector.tensor_tensor(ss[:], pr[:], cpart[:], Alu.add)
            bv = sp.tile([1, 257], dt.float32)
            nc.vector.tensor_tensor(bv[:], ss[:], pr[:], Alu.subtract)
            t4 = sp.tile([1, 257], dt.float32)
            nc.vector.tensor_tensor(t4[:], ss[:], bv[:], Alu.subtract)
            e2b = sp.tile([1, 257], dt.float32)
            nc.vector.tensor_tensor(e2b[:], pr[:], t4[:], Alu.subtract)
            e2c = sp.tile([1, 257], dt.float32)
            nc.vector.tensor_tensor(e2c[:], cpart[:], bv[:], Alu.subtract)
            e2 = sp.tile([1, 257], dt.float32)
            nc.vector.tensor_tensor(e2[:], e2b[:], e2c[:], Alu.add)
            corr = sp.tile([1, 257], dt.float32)
            nc.vector.tensor_tensor(corr[:], e2[:], er[:], Alu.add)
            edges = sp.tile([1, 257], dt.float32)
            nc.vector.tensor_tensor(edges[:], ss[:], corr[:], Alu.add)
            centers = sp.tile([1, 256], dt.float32)
            nc.vector.tensor_tensor(centers[:], edges[:, 0:256],
                                    edges[:, 1:257], Alu.add)
            nc.vector.tensor_scalar(centers[:], centers[:], 0.5, None,
                                    Alu.mult)

            # ---- Phase C: quantize -> ramp planes -> PE outer products ----
            NBANK = 4
            pt = [pp.tile([128, 128], dt.float32, name=f"pt{k}")
                  for k in range(NBANK)]
            ztp = pp.tile([16, 128], dt.float32, name="ztp")
            ysb = sp.tile([P, 128], dt.float32)
            zi32 = sp.tile([16, NBANK, 128], dt.int32)

            # engine split for the 32 ramp ops per chunk
            def ramp(eng, plane, src, i):
                if eng == "act":
                    nc.scalar.activation(plane, src,
                                         Act.Identity if i == 0 else Act.Relu,
                                         bias=biasr[:, i:i + 1], scale=1.0)
                elif i == 0:
                    if eng == "pool":
                        nc.gpsimd.tensor_scalar(plane, src, 1.0, None, Alu.add)
                    else:
                        nc.vector.tensor_scalar(plane, src, 1.0, None, Alu.add)
                else:
                    b = float(1 - i)
                    if eng == "pool":
                        nc.gpsimd.tensor_scalar(plane, src, b, 0.0,
                                                Alu.add, Alu.max)
                    else:
                        nc.vector.tensor_scalar(plane, src, b, 0.0,
                                                Alu.add, Alu.max)

            # (engine, side, plane) schedule: DVE ramps ~200ns, ACT ~614ns.
            # DVE takes 24 planes (all 16 fine + 8 coarse), ACT the other 8.
            SCHED = [("f", i) for i in range(16)] + [("c", i) for i in range(16)]
            ENG = (["dve"] * 24 + ["act"] * 8)

            with (
                tc.tile_pool(name="q", bufs=1 if dbg else 2) as qp,
                tc.tile_pool(name="planes", bufs=1 if dbg else 2) as plp,
            ):
                for ch in range(NPCH):
                    sl = slice(ch * PCH, (ch + 1) * PCH)
                    qc = qp.tile([P, PCH], dt.int16, tag="q")
                    nc.vector.tensor_scalar(qc[:], gray[:, sl], nadj[:],
                                            s256, Alu.add, Alu.mult)
                    cic = qp.tile([P, PCH], dt.int16, tag="ci")
                    nc.vector.tensor_scalar(cic[:], qc[:], 4, 15,
                                            Alu.logical_shift_right,
                                            Alu.bitwise_and)
                    fic = qp.tile([P, PCH], dt.int16, tag="fi")
                    nc.vector.tensor_scalar(fic[:], qc[:], 15, None,
                                            Alu.bitwise_and)
                    cfc = qp.tile([P, PCH], dt.bfloat16, tag="cf")
                    nc.scalar.copy(cfc[:], cic[:])
                    ffc = qp.tile([P, PCH], dt.bfloat16, tag="ff")
                    nc.scalar.copy(ffc[:], fic[:])
                    # fine side: weights-interleaved [block, plane, v] so each
                    # matmul's weights AP is one contiguous 128-element run
                    wfi = plp.tile([P, PCH // G, 16, G], dt.bfloat16, tag="rf")
                    rcp = plp.tile([P, 16, PCH], dt.bfloat16, tag="rc")
                    ffv = ffc[:].rearrange("p (b v) -> p b v", v=G)
                    for k, (side, i) in enumerate(SCHED):
                        if side == "f":
                            ramp(ENG[k], wfi[:, :, i, :], ffv, i)
                        else:
                            ramp(ENG[k], rcp[:, i, :], cfc[:], i)
                    bank = pt[ch % NBANK]
                    first, last = ch < NBANK, ch >= NPCH - NBANK
                    for v in range(0, PCH, G):
                        lw = wfi[:, v // G].rearrange("p i v -> p (i v)")
                        nc.tensor.matmul(bank[:], lhsT=lw,
                                         rhs=rcp[:, :, v:v + G],
                                         start=(first and v == 0),
                                         stop=(last and v == PCH - G))
                    if dbg and ch == NPCH - 1:
                        pf = sp.tile([1, 16, PCH], dt.float32, name="pfd")
                        nc.vector.tensor_copy(
                            pf[:], wfi[0:1].rearrange("o b i v -> o i (b v)"))
                        nc.sync.dma_start(
                            dbg_d[:, 16899:25091].rearrange(
                                "o (i v) -> o i v", i=16), pf[:])
                        pc = sp.tile([1, 16, PCH], dt.float32, name="pcd")
                        nc.vector.tensor_copy(pc[:], rcp[0:1])
                        nc.sync.dma_start(
                            dbg_d[:, 25091:33283].rearrange(
                                "o (i v) -> o i v", i=16), pc[:])
                        cfd = sp.tile([1, PCH], dt.float32, name="cfd")
                        nc.vector.tensor_copy(cfd[:], cfc[0:1])
                        nc.sync.dma_start(dbg_d[:, 33283:33795], cfd[:])
                        ffd = sp.tile([1, PCH], dt.float32, name="ffd")
                        nc.vector.tensor_copy(ffd[:], ffc[0:1])
                        nc.sync.dma_start(dbg_d[:, 33795:34307], ffd[:])
                    # bank (ch - (NPCH-NBANK)) is complete; extract diagonals
                    if ch >= NPCH - NBANK:
                        k = ch - (NPCH - NBANK)
                        nc.vector.tensor_tensor(ysb[:], pt[k][:], dmask[:],
                                                Alu.mult)
                        nc.tensor.matmul(ztp[:], lhsT=wsel[:], rhs=ysb[:],
                                         start=True, stop=True)
                        nc.vector.tensor_copy(zi32[:, k, :], ztp[:])

            if dbg:
                ptf = sp.tile([P, 128], dt.float32, name="ptf")
                nc.vector.tensor_copy(ptf[:], pt[0][:])
                nc.sync.dma_start(
                    dbg_d[:, 515:16899].rearrange(
                        "o (p n) -> (o p) n", p=P), ptf[:])
            # ---- Phase D: v+bank reduce (int32), double 2nd-difference ----
            # zi32[b, k, 8a+v] -> R[b, a] = sum over k, v
            zv = sp.tile([16, NBANK, 16], dt.int32)
            rmat = sp.tile([16, 16], dt.int32)
            with nc.allow_low_precision(reason="int32 adds are exact here"):
                nc.vector.tensor_reduce(
                    zv[:], zi32[:].rearrange("p k (a v) -> p k a v", v=G),
                    axis=Ax.X, op=Alu.add)
                nc.vector.tensor_reduce(
                    rmat[:], zv[:].rearrange("p k a -> p a k"),
                    axis=Ax.X, op=Alu.add)
            # Delta^2 over coarse (free axis): H1[:, c] = R[:,c] -2R[:,c+1] +R[:,c+2]
            h1 = sp.tile([16, 16], dt.int32)
            tsum = sp.tile([16, 14], dt.int32)
            nc.vector.tensor_tensor(tsum[:], rmat[:, 0:14], rmat[:, 2:16],
                                    Alu.add)
            nc.vector.scalar_tensor_tensor(h1[:, 0:14], rmat[:, 1:15], -2.0,
                                           tsum[:], Alu.mult, Alu.add)
            nc.vector.scalar_tensor_tensor(h1[:, 14:15], rmat[:, 15:16], -2.0,
                                           rmat[:, 14:15], Alu.mult, Alu.add)
            nc.vector.tensor_copy(h1[:, 15:16], rmat[:, 15:16])
            h1f = sp.tile([16, 16], dt.float32)
            nc.vector.tensor_copy(h1f[:], h1[:])
            # Delta^2 over fine (partition axis) via tiny constant matmul
            hps = pp.tile([16, 16], dt.float32, name="hps")
            nc.tensor.matmul(hps[:], lhsT=dmat[:], rhs=h1f[:],
                             start=True, stop=True)
            hist_sb = sp.tile([16, 16], dt.float32)
            nc.vector.tensor_copy(hist_sb[:], hps[:])
            h_in = dp.tile([1, 256], dt.float32)
            h_out = dp.tile([1, 256], dt.float32)
            nc.sync.dma_start(
                h_in[:].rearrange("o (f c) -> (o f) c", f=16), hist_sb[:])
            nc.gpsimd.collective_compute("AllReduce", Alu.add,
                                         replica_groups=groups,
                                         ins=[h_in.opt()],
                                         outs=[h_out.opt()])
            # h_out is f-major (16f + c); read back in bin order b = 16c + f
            hsb = sp.tile([1, 256], dt.float32)
            nc.sync.dma_start(
                hsb[:], h_out[:].rearrange("o (f c) -> o c f", f=16))

            # ---- Otsu on partition 0 ----
            zz = sp.tile([1, 256], dt.float32)
            nc.gpsimd.memset(zz[:], 0.0)
            w1 = sp.tile([1, 256], dt.float32)
            nc.vector.tensor_tensor_scan(w1[:], hsb[:], zz[:], 0.0,
                                         Alu.add, Alu.add)
            w2 = sp.tile([1, 256], dt.float32)
            nc.vector.tensor_tensor_scan(w2[:, ::-1], hsb[:, ::-1], zz[:],
                                         0.0, Alu.add, Alu.add)
            hc = sp.tile([1, 256], dt.float32)
            nc.vector.tensor_tensor(hc[:], hsb[:], centers[:], Alu.mult)
            s1 = sp.tile([1, 256], dt.float32)
            nc.vector.tensor_tensor_scan(s1[:], hc[:], zz[:], 0.0,
                                         Alu.add, Alu.add)
            s2 = sp.tile([1, 256], dt.float32)
            nc.vector.tensor_tensor_scan(s2[:, ::-1], hc[:, ::-1], zz[:],
                                         0.0, Alu.add, Alu.add)
            w1m = sp.tile([1, 256], dt.float32)
            nc.vector.tensor_scalar(w1m[:], w1[:], 1.0, None, Alu.max)
            w2m = sp.tile([1, 256], dt.float32)
            nc.vector.tensor_scalar(w2m[:], w2[:], 1.0, None, Alu.max)
            r1 = sp.tile([1, 256], dt.float32)
            nc.vector.reciprocal(r1[:], w1m[:])
            r2 = sp.tile([1, 256], dt.float32)
            nc.vector.reciprocal(r2[:], w2m[:])
            m1 = sp.tile([1, 256], dt.float32)
            nc.vector.tensor_tensor(m1[:], s1[:], r1[:], Alu.mult)
            m2 = sp.tile([1, 256], dt.float32)
            nc.vector.tensor_tensor(m2[:], s2[:], r2[:], Alu.mult)
            dmv = sp.tile([1, 255], dt.float32)
            nc.vector.tensor_tensor(dmv[:], m1[:, 0:255], m2[:, 1:256],
                                    Alu.subtract)
            d2 = sp.tile([1, 255], dt.float32)
            nc.vector.tensor_tensor(d2[:], dmv[:], dmv[:], Alu.mult)
            ww = sp.tile([1, 255], dt.float32)
            nc.vector.tensor_tensor(ww[:], w1[:, 0:255], w2[:, 1:256],
                                    Alu.mult)
            var = sp.tile([1, 255], dt.float32)
            nc.vector.tensor_tensor(var[:], ww[:], d2[:], Alu.mult)
            mx8 = sp.tile([1, 8], dt.float32)
            nc.vector.max(mx8[:], var[:])
            idx8 = sp.tile([1, 8], dt.uint32)
            nc.vector.max_index(idx8[:], mx8[:], var[:])
            idxf = sp.tile([1, 1], dt.float32)
            nc.vector.tensor_copy(idxf[:], idx8[:, 0:1])
            eqm = sp.tile([1, 256], dt.float32)
            nc.vector.tensor_scalar(eqm[:], iof[:, 0:256], idxf[:], None,
                                    Alu.is_equal)
            csel = sp.tile([1, 256], dt.float32)
            nc.vector.tensor_tensor(csel[:], eqm[:], centers[:], Alu.mult)
            thr11 = sp.tile([1, 1], dt.float32)
            nc.vector.tensor_reduce(thr11[:], csel[:], axis=Ax.X, op=Alu.add)
            thrb = sp.tile([P, 1], dt.float32)
            nc.gpsimd.partition_broadcast(thrb[:], thr11[:], channels=P)
            if dbg:
                nc.sync.dma_start(dbg_d[:, 0:2], mmg[:])
                nc.sync.dma_start(dbg_d[:, 2:258], hsb[:])
                nc.sync.dma_start(dbg_d[:, 258:259], thr11[:])
                rmf = sp.tile([16, 16], dt.float32)
                nc.vector.tensor_copy(rmf[:], rmat[:])
                nc.sync.dma_start(
                    dbg_d[:, 259:515].rearrange("o (b a) -> (o b) a", b=16),
                    rmf[:])
                zif = sp.tile([16, NBANK, 128], dt.float32)
                nc.vector.tensor_copy(zif[:], zi32[:])
                nc.sync.dma_start(
                    dbg_d[:, 34307:42499].rearrange(
                        "o (b k n) -> (o b) k n", b=16, k=NBANK),
                    zif[:])

            # ---- Phase E: binarize + replicate + store ----
            with tc.tile_pool(name="outp", bufs=3) as op_:
                for ch in range(NCH):
                    ot = op_.tile([P, CIN], dt.float32)
                    ov3 = ot[:].rearrange("p (v c) -> p v c", c=3)
                    gsb = gray[:, ch * CPIX:(ch + 1) * CPIX].unsqueeze(
                        2).to_broadcast((P, CPIX, 3))
                    nc.vector.tensor_scalar(ov3, gsb, thrb[:], None, Alu.is_gt)
                    nc.sync.dma_start(out_d[:, ch * CIN:(ch + 1) * CIN], ot[:])

    nc.compile()
    return nc


def get_nc():
    if "nc" not in _NC_CACHE:
        _NC_CACHE["nc"] = _build_nc()
    return _NC_CACHE["nc"]


def _shard(x):
    x = np.ascontiguousarray(x, dtype=np.float32)
    return [x[c * BPC:(c + 1) * BPC].reshape(P, FIN) for c in range(NCORES)]


def kernel(inputs):
    from concourse.bass_utils import run_bass_kernel_spmd

    nc = get_nc()
    in_maps = [{"x": s} for s in _shard(inputs)]
    res = run_bass_kernel_spmd(nc, in_maps, core_ids=list(range(NCORES)))
    out = np.concatenate(
        [res.results[c]["out"].reshape(BPC, H, WD, C) for c in range(NCORES)],
        axis=0)
    return out


# revision 23
# speedup vs baseline: 1.1765x; 1.1765x over previous
"""Otsu-threshold binarization (histogram_binning) as a Bass/Tile kernel on 8 TRN2 cores.

Pipeline per core (data-parallel over batch, shard = 4 of 32 batches):
  1. Stream input, fuse RGB->gray (fp32, DVE); per-chunk min/max on Pool.
  2. Cross-partition reduce + AllReduce(max) of [-vmin, vmax].
  3. Quantize gray to q in [0,256]; split coarse c=min(q>>4,15), fine f=q&15.
     Build 16 ReLU *ramp* planes per side (r_{-1}=x+1, r_a=relu(x-a)),
     plane-major bf16, split across DVE/ACT/Pool engines. The 16x16 joint
     histogram is the double second-difference of the ramp outer-product
     matrix R[b,a] = sum_px rf_b * rc_a, which the PE accumulates as
     block-diagonal 128x128 matmuls (8 value-slots per matmul, fp32 PSUM,
     exact: per-bank partial sums < 2^24).
  4. Diagonal extraction on-chip: mask out[8b+v, 8a+v'] with v==v' (DVE),
     column-sum via constant selector matmul (PE), v+bank reduce in int32,
     then Delta^2 over coarse (DVE) and over fine (tiny constant matmul).
  5. AllReduce(add) of the 256-bin histogram; on-chip Otsu (edges via
     emulated-FMA linspace, fp32 cumsums, argmax via Max8/MaxIndex).
  6. Binarize gray > thresh, replicate to 3 channels, stream out.
"""
import os
import numpy as np

P = 128
NCORES = 8
B, H, WD, C = 32, 128, 2048, 3
BPC = B // NCORES                  # batches per core
FIN = BPC * H * WD * C // P        # 24576 raw values per partition
FPIX = FIN // 3                    # 8192 pixels per partition
NCH = 8                            # streaming chunks
CPIX = FPIX // NCH                 # 1024 pixels per chunk
CIN = CPIX * 3                     # 3072 raw values per chunk
PCH = 512                          # histogram chunk (values per partition)
NPCH = FPIX // PCH                 # 16
G = 8                              # value-slots stacked per matmul
WR, WG, WB = 0.2989, 0.5870, 0.1140

_NC_CACHE = {}


def _build_nc():
    import concourse.mybir as mybir
    import concourse.tile as tile
    from concourse import bacc
    import bass_rust

    dt = mybir.dt
    Alu = mybir.AluOpType
    Ax = mybir.AxisListType
    Act = mybir.ActivationFunctionType
    Red = bass_rust.ReduceOp
    groups = [list(range(NCORES))]

    nc = bacc.Bacc("TRN2", target_bir_lowering=False, debug=False,
                   num_devices=NCORES)
    x_d = nc.dram_tensor("x", [P, FIN], dt.float32, kind="ExternalInput").ap()
    out_d = nc.dram_tensor("out", [P, FIN], dt.float32, kind="ExternalOutput").ap()
    dbg = os.environ.get("KDBG", "") == "1"
    if dbg:
        dbg_d = nc.dram_tensor("dbg", [1, 51200], dt.float32,
                               kind="ExternalOutput").ap()

    with tile.TileContext(nc) as tc:
        with (
            tc.tile_pool(name="gray", bufs=1) as gp,
            tc.tile_pool(name="smol", bufs=1) as sp,
            tc.tile_pool(name="cst", bufs=1) as kp,
            tc.tile_pool(name="psum", bufs=1, space="PSUM") as pp,
            tc.tile_pool(name="dram", bufs=1, space="DRAM") as dp,
        ):
            gray = gp.tile([P, FPIX], dt.float32)
            mns = sp.tile([P, NCH], dt.float32)
            mxs = sp.tile([P, NCH], dt.float32)

            # ---- constants (independent; overlap with input DMA) ----
            # diag mask D[p, n] = 1{(n - p) % 8 == 0}  (v == v')
            ti = kp.tile([P, 128], dt.int32)
            nc.gpsimd.iota(ti[:], pattern=[[1, 128]], base=0,
                           channel_multiplier=-1)
            tand = kp.tile([P, 128], dt.int32)
            nc.vector.tensor_scalar(tand[:], ti[:], 7, None, Alu.bitwise_and)
            dmask = kp.tile([P, 128], dt.float32)
            nc.vector.tensor_scalar(dmask[:], tand[:], 0, None, Alu.is_equal)
            # selector W[p, m] = 1{m == p >> 3}
            prow = kp.tile([P, 16], dt.int32)
            nc.gpsimd.iota(prow[:], pattern=[[0, 16]], base=0,
                           channel_multiplier=1)
            prs = kp.tile([P, 16], dt.int32)
            nc.vector.tensor_scalar(prs[:], prow[:], 3, None,
                                    Alu.logical_shift_right)
            col16 = kp.tile([P, 16], dt.int32)
            nc.gpsimd.iota(col16[:], pattern=[[1, 16]], base=0,
                           channel_multiplier=0)
            wsel = kp.tile([P, 16], dt.float32)
            nc.vector.tensor_tensor(wsel[:], prs[:], col16[:], Alu.is_equal)
            # second-difference matrix Dm[k, m] = +1{k==m} -2{k==m+1} +1{k==m+2}
            t16 = kp.tile([16, 16], dt.int32)
            nc.gpsimd.iota(t16[:], pattern=[[1, 16]], base=0,
                           channel_multiplier=-1)
            dm0 = kp.tile([16, 16], dt.float32)
            nc.vector.tensor_scalar(dm0[:], t16[:], 0, None, Alu.is_equal)
            dm2 = kp.tile([16, 16], dt.float32)
            nc.vector.tensor_scalar(dm2[:], t16[:], -2, None, Alu.is_equal)
            dm1 = kp.tile([16, 16], dt.float32)
            nc.vector.tensor_scalar(dm1[:], t16[:], -1, None, Alu.is_equal)
            dmat = kp.tile([16, 16], dt.float32)
            nc.vector.tensor_tensor(dmat[:], dm0[:], dm2[:], Alu.add)
            nc.vector.scalar_tensor_tensor(dmat[:], dm1[:], -2.0, dmat[:],
                                           Alu.mult, Alu.add)
            # ramp biases for ACT: column i holds 1 - i
            bia = kp.tile([P, 16], dt.int32)
            nc.gpsimd.iota(bia[:], pattern=[[1, 16]], base=0,
                           channel_multiplier=0)
            biasr = kp.tile([P, 16], dt.float32)
            nc.vector.tensor_scalar(biasr[:], bia[:], -1.0, 1.0,
                                    Alu.mult, Alu.add)
            # iota ramp for otsu (t in [0,1])
            io32 = kp.tile([1, 257], dt.int32)
            nc.gpsimd.iota(io32[:], pattern=[[1, 257]], base=0,
                           channel_multiplier=0)
            iof = kp.tile([1, 257], dt.float32)
            nc.vector.tensor_copy(iof[:], io32[:])

            # ---- Phase A: load + grayscale + per-chunk min/max (Pool) ----
            with tc.tile_pool(name="xin", bufs=3) as inp:
                for ch in range(NCH):
                    xt = inp.tile([P, CIN], dt.float32)
                    nc.sync.dma_start(xt[:], x_d[:, ch * CIN:(ch + 1) * CIN])
                    xv = xt[:].rearrange("p (v c) -> p c v", c=3)
                    gs = gray[:, ch * CPIX:(ch + 1) * CPIX]
                    nc.vector.tensor_scalar(gs, xv[:, 0], WR, None, Alu.mult)
                    nc.vector.scalar_tensor_tensor(gs, xv[:, 1], WG, gs,
                                                   Alu.mult, Alu.add)
                    nc.vector.scalar_tensor_tensor(gs, xv[:, 2], WB, gs,
                                                   Alu.mult, Alu.add)
                    nc.vector.tensor_reduce(mns[:, ch:ch + 1], gs, axis=Ax.X,
                                            op=Alu.min)
                    nc.vector.tensor_reduce(mxs[:, ch:ch + 1], gs, axis=Ax.X,
                                            op=Alu.max)

            # ---- Phase B: global min/max ----
            mn = sp.tile([P, 1], dt.float32)
            mx = sp.tile([P, 1], dt.float32)
            nc.vector.tensor_reduce(mn, mns[:], axis=Ax.X, op=Alu.min)
            nc.vector.tensor_reduce(mx, mxs[:], axis=Ax.X, op=Alu.max)
            mm2 = sp.tile([P, 2], dt.float32)   # [-vmin, vmax]
            nc.vector.tensor_scalar(mm2[:, 0:1], mn[:], -1.0, None, Alu.mult)
            nc.vector.tensor_copy(mm2[:, 1:2], mx[:])
            mmr = sp.tile([P, 2], dt.float32)
            nc.gpsimd.partition_all_reduce(mmr[:], mm2[:], channels=P,
                                           reduce_op=Red.max)
            mm_in = dp.tile([1, 2], dt.float32)
            mm_out = dp.tile([1, 2], dt.float32)
            nc.sync.dma_start(mm_in[:], mmr[0:1, :])
            nc.gpsimd.collective_compute("AllReduce", Alu.max,
                                         replica_groups=groups,
                                         ins=[mm_in.opt()],
                                         outs=[mm_out.opt()])
            mmg = sp.tile([1, 2], dt.float32)
            nc.sync.dma_start(mmg[:], mm_out[:])
            mmb = sp.tile([P, 2], dt.float32)  # [:,0] = -vmin, [:,1] = vmax
            nc.gpsimd.partition_broadcast(mmb[:], mmg[:], channels=P)
            negvmin = mmb[:, 0:1]
            vmaxc = mmb[:, 1:2]
            delta = sp.tile([P, 1], dt.float32)
            nc.vector.tensor_tensor(delta[:], vmaxc, negvmin, Alu.add)
            rdel = sp.tile([P, 1], dt.float32)
            nc.vector.reciprocal(rdel[:], delta[:])
            s256 = sp.tile([P, 1], dt.float32)
            nc.vector.tensor_scalar(s256[:], rdel[:], 256.0, None, Alu.mult)
            # HW float->int converts round-to-nearest; pre-subtract half a
            # bin so round(y - 0.5) == trunc(y): A = -vmin - delta/512
            hstep = sp.tile([P, 1], dt.float32)
            nc.vector.tensor_scalar(hstep[:], delta[:], 1.0 / 512.0, None,
                                    Alu.mult)
            nadj = sp.tile([P, 1], dt.float32)
            nc.vector.tensor_tensor(nadj[:], negvmin, hstep[:], Alu.subtract)

            # ---- edges/centers (depend only on mmg; overlap with Phase C) ----
            tt_ = sp.tile([1, 257], dt.float32)
            nc.vector.tensor_scalar(tt_[:], iof[:], 1.0 / 256.0, None,
                                    Alu.mult)
            omt = sp.tile([1, 257], dt.float32)
            nc.vector.tensor_scalar(omt[:], tt_[:], -1.0, 1.0, Alu.mult,
                                    Alu.add)
            vminp = sp.tile([1, 1], dt.float32)
            nc.vector.tensor_scalar(vminp[:], mmg[:, 0:1], -1.0, None,
                                    Alu.mult)
            cpart = sp.tile([1, 257], dt.float32)
            nc.vector.tensor_scalar(cpart[:], omt[:], vminp[:], None,
                                    Alu.mult)
            vx = mmg[:, 1:2]
            # emulated fma(vmax, t, cpart): Veltkamp split + 2Sum
            c1 = sp.tile([1, 1], dt.float32)
            nc.vector.tensor_scalar(c1[:], vx, 4097.0, None, Alu.mult)
            cm = sp.tile([1, 1], dt.float32)
            nc.vector.tensor_tensor(cm[:], c1[:], vx, Alu.subtract)
            ahi = sp.tile([1, 1], dt.float32)
            nc.vector.tensor_tensor(ahi[:], c1[:], cm[:], Alu.subtract)
            alo = sp.tile([1, 1], dt.float32)
            nc.vector.tensor_tensor(alo[:], vx, ahi[:], Alu.subtract)
            pr = sp.tile([1, 257], dt.float32)
            nc.vector.tensor_scalar(pr[:], tt_[:], vx, None, Alu.mult)
            hh = sp.tile([1, 257], dt.float32)
            nc.vector.tensor_scalar(hh[:], tt_[:], ahi[:], None, Alu.mult)
            e0 = sp.tile([1, 257], dt.float32)
            nc.vector.tensor_tensor(e0[:], hh[:], pr[:], Alu.subtract)
            ll = sp.tile([1, 257], dt.float32)
            nc.vector.tensor_scalar(ll[:], tt_[:], alo[:], None, Alu.mult)
            er = sp.tile([1, 257], dt.float32)
            nc.vector.tensor_tensor(er[:], e0[:], ll[:], Alu.add)
            ss = sp.tile([1, 257], dt.float32)
            nc.vector.tensor_tensor(ss[:], pr[:], cpart[:], Alu.add)
            bv = sp.tile([1, 257], dt.float32)
            nc.vector.tensor_tensor(bv[:], ss[:], pr[:], Alu.subtract)
            t4 = sp.tile([1, 257], dt.float32)
            nc.vector.tensor_tensor(t4[:], ss[:], bv[:], Alu.subtract)
            e2b = sp.tile([1, 257], dt.float32)
            nc.vector.tensor_tensor(e2b[:], pr[:], t4[:], Alu.subtract)
            e2c = sp.tile([1, 257], dt.float32)
            nc.vector.tensor_tensor(e2c[:], cpart[:], bv[:], Alu.subtract)
            e2 = sp.tile([1, 257], dt.float32)
            nc.vector.tensor_tensor(e2[:], e2b[:], e2c[:], Alu.add)
            corr = sp.tile([1, 257], dt.float32)
            nc.vector.tensor_tensor(corr[:], e2[:], er[:], Alu.add)
            edges = sp.tile([1, 257], dt.float32)
            nc.vector.tensor_tensor(edges[:], ss[:], corr[:], Alu.add)
            centers = sp.tile([1, 256], dt.float32)
            nc.vector.tensor_tensor(centers[:], edges[:, 0:256],
                                    edges[:, 1:257], Alu.add)
            nc.vector.tensor_scalar(centers[:], centers[:], 0.5, None,
                                    Alu.mult)

            # ---- Phase C: quantize -> ramp planes -> PE outer products ----
            NBANK = 4
            pt = [pp.tile([128, 128], dt.float32, name=f"pt{k}")
                  for k in range(NBANK)]
            ztp = pp.tile([16, 128], dt.float32, name="ztp")
            ysb = sp.tile([P, 128], dt.float32)
            zi32 = sp.tile([16, NBANK, 128], dt.int32)

            # engine split for the 32 ramp ops per chunk
            def ramp(eng, plane, src, i):
                if eng == "act":
                    nc.scalar.activation(plane, src,
                                         Act.Identity if i == 0 else Act.Relu,
                                         bias=biasr[:, i:i + 1], scale=1.0)
                elif i == 0:
                    if eng == "pool":
                        nc.gpsimd.tensor_scalar(plane, src, 1.0, None, Alu.add)
                    else:
                        nc.vector.tensor_scalar(plane, src, 1.0, None, Alu.add)
                else:
                    b = float(1 - i)
                    if eng == "pool":
                        nc.gpsimd.tensor_scalar(plane, src, b, 0.0,
                                                Alu.add, Alu.max)
                    else:
                        nc.vector.tensor_scalar(plane, src, b, 0.0,
                                                Alu.add, Alu.max)

            # (engine, side, plane) schedule: DVE ramps ~200ns, ACT ~614ns.
            # DVE takes 24 planes (all 16 fine + 8 coarse), ACT the other 8.
            SCHED = [("f", i) for i in range(16)] + [("c", i) for i in range(16)]
            ENG = (["dve"] * 24 + ["act"] * 8)

            with (
                tc.tile_pool(name="q", bufs=1 if dbg else 2) as qp,
                tc.tile_pool(name="planes", bufs=1 if dbg else 2) as plp,
            ):
                for ch in range(NPCH):
                    sl = slice(ch * PCH, (ch + 1) * PCH)
                    qc = qp.tile([P, PCH], dt.int16, tag="q")
                    nc.vector.tensor_scalar(qc[:], gray[:, sl], nadj[:],
                                            s256, Alu.add, Alu.mult)
                    cic = qp.tile([P, PCH], dt.int16, tag="ci")
                    nc.vector.tensor_scalar(cic[:], qc[:], 4, 15,
                                            Alu.logical_shift_right,
                                            Alu.bitwise_and)
                    fic = qp.tile([P, PCH], dt.int16, tag="fi")
                    nc.vector.tensor_scalar(fic[:], qc[:], 15, None,
                                            Alu.bitwise_and)
                    cfc = qp.tile([P, PCH], dt.bfloat16, tag="cf")
                    nc.scalar.copy(cfc[:], cic[:])
                    ffc = qp.tile([P, PCH], dt.bfloat16, tag="ff")
                    nc.scalar.copy(ffc[:], fic[:])
                    # fine side: weights-interleaved [block, plane, v] so each
                    # matmul's weights AP is one contiguous 128-element run
                    wfi = plp.tile([P, PCH // G, 16, G], dt.bfloat16, tag="rf")
                    rcp = plp.tile([P, 16, PCH], dt.bfloat16, tag="rc")
                    ffv = ffc[:].rearrange("p (b v) -> p b v", v=G)
                    for k, (side, i) in enumerate(SCHED):
                        if side == "f":
                            ramp(ENG[k], wfi[:, :, i, :], ffv, i)
                        else:
                            ramp(ENG[k], rcp[:, i, :], cfc[:], i)
                    bank = pt[ch % NBANK]
                    first, last = ch < NBANK, ch >= NPCH - NBANK
                    for v in range(0, PCH, G):
                        lw = wfi[:, v // G].rearrange("p i v -> p (i v)")
                        nc.tensor.matmul(bank[:], lhsT=lw,
                                         rhs=rcp[:, :, v:v + G],
                                         start=(first and v == 0),
                                         stop=(last and v == PCH - G))
                    if dbg and ch == NPCH - 1:
                        pf = sp.tile([1, 16, PCH], dt.float32, name="pfd")
                        nc.vector.tensor_copy(
                            pf[:], wfi[0:1].rearrange("o b i v -> o i (b v)"))
                        nc.sync.dma_start(
                            dbg_d[:, 16899:25091].rearrange(
                                "o (i v) -> o i v", i=16), pf[:])
                        pc = sp.tile([1, 16, PCH], dt.float32, name="pcd")
                        nc.vector.tensor_copy(pc[:], rcp[0:1])
                        nc.sync.dma_start(
                            dbg_d[:, 25091:33283].rearrange(
                                "o (i v) -> o i v", i=16), pc[:])
                        cfd = sp.tile([1, PCH], dt.float32, name="cfd")
                        nc.vector.tensor_copy(cfd[:], cfc[0:1])
                        nc.sync.dma_start(dbg_d[:, 33283:33795], cfd[:])
                        ffd = sp.tile([1, PCH], dt.float32, name="ffd")
                        nc.vector.tensor_copy(ffd[:], ffc[0:1])
                        nc.sync.dma_start(dbg_d[:, 33795:34307], ffd[:])
                    # bank (ch - (NPCH-NBANK)) is complete; extract diagonals
                    if ch >= NPCH - NBANK:
                        k = ch - (NPCH - NBANK)
                        nc.vector.tensor_tensor(ysb[:], pt[k][:], dmask[:],
                                                Alu.mult)
                        nc.tensor.matmul(ztp[:], lhsT=wsel[:], rhs=ysb[:],
                                         start=True, stop=True)
                        nc.vector.tensor_copy(zi32[:, k, :], ztp[:])

            if dbg:
                ptf = sp.tile([P, 128], dt.float32, name="ptf")
                nc.vector.tensor_copy(ptf[:], pt[0][:])
                nc.sync.dma_start(
                    dbg_d[:, 515:16899].rearrange(
                        "o (p n) -> (o p) n", p=P), ptf[:])
            # ---- Phase D: v+bank reduce (int32), double 2nd-difference ----
            # zi32[b, k, 8a+v] -> R[b, a] = sum over k, v
            zv = sp.tile([16, NBANK, 16], dt.int32)
            rmat = sp.tile([16, 16], dt.int32)
            with nc.allow_low_precision(reason="int32 adds are exact here"):
                nc.vector.tensor_reduce(
                    zv[:], zi32[:].rearrange("p k (a v) -> p k a v", v=G),
                    axis=Ax.X, op=Alu.add)
                nc.vector.tensor_reduce(
                    rmat[:], zv[:].rearrange("p k a -> p a k"),
                    axis=Ax.X, op=Alu.add)
            # Delta^2 over coarse (free axis): H1[:, c] = R[:,c] -2R[:,c+1] +R[:,c+2]
            h1 = sp.tile([16, 16], dt.int32)
            tsum = sp.tile([16, 14], dt.int32)
            nc.vector.tensor_tensor(tsum[:], rmat[:, 0:14], rmat[:, 2:16],
                                    Alu.add)
            nc.vector.scalar_tensor_tensor(h1[:, 0:14], rmat[:, 1:15], -2.0,
                                           tsum[:], Alu.mult, Alu.add)
            nc.vector.scalar_tensor_tensor(h1[:, 14:15], rmat[:, 15:16], -2.0,
                                           rmat[:, 14:15], Alu.mult, Alu.add)
            nc.vector.tensor_copy(h1[:, 15:16], rmat[:, 15:16])
            h1f = sp.tile([16, 16], dt.float32)
            nc.vector.tensor_copy(h1f[:], h1[:])
            # Delta^2 over fine (partition axis) via tiny constant matmul
            hps = pp.tile([16, 16], dt.float32, name="hps")
            nc.tensor.matmul(hps[:], lhsT=dmat[:], rhs=h1f[:],
                             start=True, stop=True)
            hist_sb = sp.tile([16, 16], dt.float32)
            nc.vector.tensor_copy(hist_sb[:], hps[:])
            h_in = dp.tile([1, 256], dt.float32)
            h_out = dp.tile([1, 256], dt.float32)
            nc.sync.dma_start(
                h_in[:].rearrange("o (f c) -> (o f) c", f=16), hist_sb[:])
            nc.gpsimd.collective_compute("AllReduce", Alu.add,
                                         replica_groups=groups,
                                         ins=[h_in.opt()],
                                         outs=[h_out.opt()])
            # h_out is f-major (16f + c); read back in bin order b = 16c + f
            hsb = sp.tile([1, 256], dt.float32)
            nc.sync.dma_start(
                hsb[:], h_out[:].rearrange("o (f c) -> o c f", f=16))

            # ---- Otsu on partition 0 ----
            zz = sp.tile([1, 256], dt.float32)
            nc.gpsimd.memset(zz[:], 0.0)
            w1 = sp.tile([1, 256], dt.float32)
            nc.vector.tensor_tensor_scan(w1[:], hsb[:], zz[:], 0.0,
                                         Alu.add, Alu.add)
            w2 = sp.tile([1, 256], dt.float32)
            nc.vector.tensor_tensor_scan(w2[:, ::-1], hsb[:, ::-1], zz[:],
                                         0.0, Alu.add, Alu.add)
            hc = sp.tile([1, 256], dt.float32)
            nc.vector.tensor_tensor(hc[:], hsb[:], centers[:], Alu.mult)
            s1 = sp.tile([1, 256], dt.float32)
            nc.vector.tensor_tensor_scan(s1[:], hc[:], zz[:], 0.0,
                                         Alu.add, Alu.add)
            s2 = sp.tile([1, 256], dt.float32)
            nc.vector.tensor_tensor_scan(s2[:, ::-1], hc[:, ::-1], zz[:],
                                         0.0, Alu.add, Alu.add)
            w1m = sp.tile([1, 256], dt.float32)
            nc.vector.tensor_scalar(w1m[:], w1[:], 1.0, None, Alu.max)
            w2m = sp.tile([1, 256], dt.float32)
            nc.vector.tensor_scalar(w2m[:], w2[:], 1.0, None, Alu.max)
            r1 = sp.tile([1, 256], dt.float32)
            nc.vector.reciprocal(r1[:], w1m[:])
            r2 = sp.tile([1, 256], dt.float32)
            nc.vector.reciprocal(r2[:], w2m[:])
            m1 = sp.tile([1, 256], dt.float32)
            nc.vector.tensor_tensor(m1[:], s1[:], r1[:], Alu.mult)
            m2 = sp.tile([1, 256], dt.float32)
            nc.vector.tensor_tensor(m2[:], s2[:], r2[:], Alu.mult)
            dmv = sp.tile([1, 255], dt.float32)
            nc.vector.tensor_tensor(dmv[:], m1[:, 0:255], m2[:, 1:256],
                                    Alu.subtract)
            d2 = sp.tile([1, 255], dt.float32)
            nc.vector.tensor_tensor(d2[:], dmv[:], dmv[:], Alu.mult)
            ww = sp.tile([1, 255], dt.float32)
            nc.vector.tensor_tensor(ww[:], w1[:, 0:255], w2[:, 1:256],
                                    Alu.mult)
            var = sp.tile([1, 255], dt.float32)
            nc.vector.tensor_tensor(var[:], ww[:], d2[:], Alu.mult)
            mx8 = sp.tile([1, 8], dt.float32)
            nc.vector.max(mx8[:], var[:])
            idx8 = sp.tile([1, 8], dt.uint32)
            nc.vector.max_index(idx8[:], mx8[:], var[:])
            idxf = sp.tile([1, 1], dt.float32)
            nc.vector.tensor_copy(idxf[:], idx8[:, 0:1])
            eqm = sp.tile([1, 256], dt.float32)
            nc.vector.tensor_scalar(eqm[:], iof[:, 0:256], idxf[:], None,
                                    Alu.is_equal)
            csel = sp.tile([1, 256], dt.float32)
            nc.vector.tensor_tensor(csel[:], eqm[:], centers[:], Alu.mult)
            thr11 = sp.tile([1, 1], dt.float32)
            nc.vector.tensor_reduce(thr11[:], csel[:], axis=Ax.X, op=Alu.add)
            thrb = sp.tile([P, 1], dt.float32)
            nc.gpsimd.partition_broadcast(thrb[:], thr11[:], channels=P)
            if dbg:
                nc.sync.dma_start(dbg_d[:, 0:2], mmg[:])
                nc.sync.dma_start(dbg_d[:, 2:258], hsb[:])
                nc.sync.dma_start(dbg_d[:, 258:259], thr11[:])
                rmf = sp.tile([16, 16], dt.float32)
                nc.vector.tensor_copy(rmf[:], rmat[:])
                nc.sync.dma_start(
                    dbg_d[:, 259:515].rearrange("o (b a) -> (o b) a", b=16),
                    rmf[:])
                zif = sp.tile([16, NBANK, 128], dt.float32)
                nc.vector.tensor_copy(zif[:], zi32[:])
                nc.sync.dma_start(
                    dbg_d[:, 34307:42499].rearrange(
                        "o (b k n) -> (o b) k n", b=16, k=NBANK),
                    zif[:])

            # ---- Phase E: binarize + replicate + store ----
            with tc.tile_pool(name="outp", bufs=3) as op_:
                for ch in range(NCH):
                    ot = op_.tile([P, CIN], dt.float32)
                    ov3 = ot[:].rearrange("p (v c) -> p v c", c=3)
                    gsb = gray[:, ch * CPIX:(ch + 1) * CPIX].unsqueeze(
                        2).to_broadcast((P, CPIX, 3))
                    nc.vector.tensor_scalar(ov3, gsb, thrb[:], None, Alu.is_gt)
                    nc.sync.dma_start(out_d[:, ch * CIN:(ch + 1) * CIN], ot[:])

    nc.compile()
    return nc


def get_nc():
    if "nc" not in _NC_CACHE:
        _NC_CACHE["nc"] = _build_nc()
    return _NC_CACHE["nc"]


def _shard(x):
    x = np.ascontiguousarray(x, dtype=np.float32)
    return [x[c * BPC:(c + 1) * BPC].reshape(P, FIN) for c in range(NCORES)]


def kernel(inputs):
    from concourse.bass_utils import run_bass_kernel_spmd

    nc = get_nc()
    in_maps = [{"x": s} for s in _shard(inputs)]
    res = run_bass_kernel_spmd(nc, in_maps, core_ids=list(range(NCORES)))
    out = np.concatenate(
        [res.results[c]["out"].reshape(BPC, H, WD, C) for c in range(NCORES)],
        axis=0)
    return out


# revision 24
# speedup vs baseline: 1.2553x; 1.0669x over previous
"""Otsu-threshold binarization (histogram_binning) as a Bass/Tile kernel on 8 TRN2 cores.

Pipeline per core (data-parallel over batch, shard = 4 of 32 batches):
  1. Stream input, fuse RGB->gray (fp32, DVE); per-chunk min/max on Pool.
  2. Cross-partition reduce + AllReduce(max) of [-vmin, vmax].
  3. Quantize gray to q in [0,256]; split coarse c=min(q>>4,15), fine f=q&15.
     Build 16 ReLU *ramp* planes per side (r_{-1}=x+1, r_a=relu(x-a)),
     plane-major bf16, split across DVE/ACT/Pool engines. The 16x16 joint
     histogram is the double second-difference of the ramp outer-product
     matrix R[b,a] = sum_px rf_b * rc_a, which the PE accumulates as
     block-diagonal 128x128 matmuls (8 value-slots per matmul, fp32 PSUM,
     exact: per-bank partial sums < 2^24).
  4. Diagonal extraction on-chip: mask out[8b+v, 8a+v'] with v==v' (DVE),
     column-sum via constant selector matmul (PE), v+bank reduce in int32,
     then Delta^2 over coarse (DVE) and over fine (tiny constant matmul).
  5. AllReduce(add) of the 256-bin histogram; on-chip Otsu (edges via
     emulated-FMA linspace, fp32 cumsums, argmax via Max8/MaxIndex).
  6. Binarize gray > thresh, replicate to 3 channels, stream out.
"""
import os
import numpy as np

P = 128
NCORES = 8
B, H, WD, C = 32, 128, 2048, 3
BPC = B // NCORES                  # batches per core
FIN = BPC * H * WD * C // P        # 24576 raw values per partition
FPIX = FIN // 3                    # 8192 pixels per partition
NCH = 8                            # streaming chunks
CPIX = FPIX // NCH                 # 1024 pixels per chunk
CIN = CPIX * 3                     # 3072 raw values per chunk
PCH = 512                          # histogram chunk (values per partition)
NPCH = FPIX // PCH                 # 16
G = 8                              # value-slots stacked per matmul
WR, WG, WB = 0.2989, 0.5870, 0.1140

_NC_CACHE = {}


def _build_nc():
    import concourse.mybir as mybir
    import concourse.tile as tile
    from concourse import bacc
    import bass_rust

    dt = mybir.dt
    Alu = mybir.AluOpType
    Ax = mybir.AxisListType
    Act = mybir.ActivationFunctionType
    Red = bass_rust.ReduceOp
    groups = [list(range(NCORES))]

    nc = bacc.Bacc("TRN2", target_bir_lowering=False, debug=False,
                   num_devices=NCORES)
    x_d = nc.dram_tensor("x", [P, FIN], dt.float32, kind="ExternalInput").ap()
    out_d = nc.dram_tensor("out", [P, FIN], dt.float32, kind="ExternalOutput").ap()
    dbg = os.environ.get("KDBG", "") == "1"
    if dbg:
        dbg_d = nc.dram_tensor("dbg", [1, 51200], dt.float32,
                               kind="ExternalOutput").ap()

    with tile.TileContext(nc) as tc:
        with (
            tc.tile_pool(name="gray", bufs=1) as gp,
            tc.tile_pool(name="smol", bufs=1) as sp,
            tc.tile_pool(name="cst", bufs=1) as kp,
            tc.tile_pool(name="psum", bufs=1, space="PSUM") as pp,
            tc.tile_pool(name="dram", bufs=1, space="DRAM") as dp,
        ):
            gray = gp.tile([P, FPIX], dt.float32)
            mns = sp.tile([P, NCH], dt.float32)
            mxs = sp.tile([P, NCH], dt.float32)

            # ---- constants (independent; overlap with input DMA) ----
            # diag mask D[p, n] = 1{(n - p) % 8 == 0}  (v == v')
            ti = kp.tile([P, 128], dt.int32)
            nc.gpsimd.iota(ti[:], pattern=[[1, 128]], base=0,
                           channel_multiplier=-1)
            tand = kp.tile([P, 128], dt.int32)
            nc.vector.tensor_scalar(tand[:], ti[:], 7, None, Alu.bitwise_and)
            dmask = kp.tile([P, 128], dt.float32)
            nc.vector.tensor_scalar(dmask[:], tand[:], 0, None, Alu.is_equal)
            # selector W[p, m] = 1{m == p >> 3}
            prow = kp.tile([P, 16], dt.int32)
            nc.gpsimd.iota(prow[:], pattern=[[0, 16]], base=0,
                           channel_multiplier=1)
            prs = kp.tile([P, 16], dt.int32)
            nc.vector.tensor_scalar(prs[:], prow[:], 3, None,
                                    Alu.logical_shift_right)
            col16 = kp.tile([P, 16], dt.int32)
            nc.gpsimd.iota(col16[:], pattern=[[1, 16]], base=0,
                           channel_multiplier=0)
            wsel = kp.tile([P, 16], dt.float32)
            nc.vector.tensor_tensor(wsel[:], prs[:], col16[:], Alu.is_equal)
            # second-difference matrix Dm[k, m] = +1{k==m} -2{k==m+1} +1{k==m+2}
            t16 = kp.tile([16, 16], dt.int32)
            nc.gpsimd.iota(t16[:], pattern=[[1, 16]], base=0,
                           channel_multiplier=-1)
            dm0 = kp.tile([16, 16], dt.float32)
            nc.vector.tensor_scalar(dm0[:], t16[:], 0, None, Alu.is_equal)
            dm2 = kp.tile([16, 16], dt.float32)
            nc.vector.tensor_scalar(dm2[:], t16[:], -2, None, Alu.is_equal)
            dm1 = kp.tile([16, 16], dt.float32)
            nc.vector.tensor_scalar(dm1[:], t16[:], -1, None, Alu.is_equal)
            dmat = kp.tile([16, 16], dt.float32)
            nc.vector.tensor_tensor(dmat[:], dm0[:], dm2[:], Alu.add)
            nc.vector.scalar_tensor_tensor(dmat[:], dm1[:], -2.0, dmat[:],
                                           Alu.mult, Alu.add)
            # ramp biases for ACT: column i holds 1 - i
            bia = kp.tile([P, 16], dt.int32)
            nc.gpsimd.iota(bia[:], pattern=[[1, 16]], base=0,
                           channel_multiplier=0)
            biasr = kp.tile([P, 16], dt.float32)
            nc.vector.tensor_scalar(biasr[:], bia[:], -1.0, 1.0,
                                    Alu.mult, Alu.add)
            # iota ramp for otsu (t in [0,1])
            io32 = kp.tile([1, 257], dt.int32)
            nc.gpsimd.iota(io32[:], pattern=[[1, 257]], base=0,
                           channel_multiplier=0)
            iof = kp.tile([1, 257], dt.float32)
            nc.vector.tensor_copy(iof[:], io32[:])

            # ---- Phase A: load + grayscale + per-chunk min/max (Pool) ----
            with tc.tile_pool(name="xin", bufs=4) as inp:
                for ch in range(NCH):
                    xt = inp.tile([P, CIN], dt.float32)
                    nc.sync.dma_start(xt[:], x_d[:, ch * CIN:(ch + 1) * CIN])
                    xv = xt[:].rearrange("p (v c) -> p c v", c=3)
                    gs = gray[:, ch * CPIX:(ch + 1) * CPIX]
                    nc.vector.tensor_scalar(gs, xv[:, 0], WR, None, Alu.mult)
                    nc.vector.scalar_tensor_tensor(gs, xv[:, 1], WG, gs,
                                                   Alu.mult, Alu.add)
                    nc.vector.scalar_tensor_tensor(gs, xv[:, 2], WB, gs,
                                                   Alu.mult, Alu.add)
                    nc.vector.tensor_reduce(mns[:, ch:ch + 1], gs, axis=Ax.X,
                                            op=Alu.min)
                    nc.vector.tensor_reduce(mxs[:, ch:ch + 1], gs, axis=Ax.X,
                                            op=Alu.max)

            # ---- Phase B: global min/max ----
            mn = sp.tile([P, 1], dt.float32)
            mx = sp.tile([P, 1], dt.float32)
            nc.vector.tensor_reduce(mn, mns[:], axis=Ax.X, op=Alu.min)
            nc.vector.tensor_reduce(mx, mxs[:], axis=Ax.X, op=Alu.max)
            mm2 = sp.tile([P, 2], dt.float32)   # [-vmin, vmax]
            nc.vector.tensor_scalar(mm2[:, 0:1], mn[:], -1.0, None, Alu.mult)
            nc.vector.tensor_copy(mm2[:, 1:2], mx[:])
            mmr = sp.tile([P, 2], dt.float32)
            nc.gpsimd.partition_all_reduce(mmr[:], mm2[:], channels=P,
                                           reduce_op=Red.max)
            mm_in = dp.tile([1, 2], dt.float32)
            mm_out = dp.tile([1, 2], dt.float32)
            nc.sync.dma_start(mm_in[:], mmr[0:1, :])
            nc.gpsimd.collective_compute("AllReduce", Alu.max,
                                         replica_groups=groups,
                                         ins=[mm_in.opt()],
                                         outs=[mm_out.opt()])
            mmg = sp.tile([1, 2], dt.float32)
            nc.sync.dma_start(mmg[:], mm_out[:])
            mmb = sp.tile([P, 2], dt.float32)  # [:,0] = -vmin, [:,1] = vmax
            nc.gpsimd.partition_broadcast(mmb[:], mmg[:], channels=P)
            negvmin = mmb[:, 0:1]
            vmaxc = mmb[:, 1:2]
            delta = sp.tile([P, 1], dt.float32)
            nc.vector.tensor_tensor(delta[:], vmaxc, negvmin, Alu.add)
            rdel = sp.tile([P, 1], dt.float32)
            nc.vector.reciprocal(rdel[:], delta[:])
            s256 = sp.tile([P, 1], dt.float32)
            nc.vector.tensor_scalar(s256[:], rdel[:], 256.0, None, Alu.mult)
            # HW float->int converts round-to-nearest; pre-subtract half a
            # bin so round(y - 0.5) == trunc(y): A = -vmin - delta/512
            hstep = sp.tile([P, 1], dt.float32)
            nc.vector.tensor_scalar(hstep[:], delta[:], 1.0 / 512.0, None,
                                    Alu.mult)
            nadj = sp.tile([P, 1], dt.float32)
            nc.vector.tensor_tensor(nadj[:], negvmin, hstep[:], Alu.subtract)

            # ---- edges/centers (depend only on mmg; overlap with Phase C) ----
            tt_ = sp.tile([1, 257], dt.float32)
            nc.vector.tensor_scalar(tt_[:], iof[:], 1.0 / 256.0, None,
                                    Alu.mult)
            omt = sp.tile([1, 257], dt.float32)
            nc.vector.tensor_scalar(omt[:], tt_[:], -1.0, 1.0, Alu.mult,
                                    Alu.add)
            vminp = sp.tile([1, 1], dt.float32)
            nc.vector.tensor_scalar(vminp[:], mmg[:, 0:1], -1.0, None,
                                    Alu.mult)
            cpart = sp.tile([1, 257], dt.float32)
            nc.vector.tensor_scalar(cpart[:], omt[:], vminp[:], None,
                                    Alu.mult)
            vx = mmg[:, 1:2]
            # emulated fma(vmax, t, cpart): Veltkamp split + 2Sum
            c1 = sp.tile([1, 1], dt.float32)
            nc.vector.tensor_scalar(c1[:], vx, 4097.0, None, Alu.mult)
            cm = sp.tile([1, 1], dt.float32)
            nc.vector.tensor_tensor(cm[:], c1[:], vx, Alu.subtract)
            ahi = sp.tile([1, 1], dt.float32)
            nc.vector.tensor_tensor(ahi[:], c1[:], cm[:], Alu.subtract)
            alo = sp.tile([1, 1], dt.float32)
            nc.vector.tensor_tensor(alo[:], vx, ahi[:], Alu.subtract)
            pr = sp.tile([1, 257], dt.float32)
            nc.vector.tensor_scalar(pr[:], tt_[:], vx, None, Alu.mult)
            hh = sp.tile([1, 257], dt.float32)
            nc.vector.tensor_scalar(hh[:], tt_[:], ahi[:], None, Alu.mult)
            e0 = sp.tile([1, 257], dt.float32)
            nc.vector.tensor_tensor(e0[:], hh[:], pr[:], Alu.subtract)
            ll = sp.tile([1, 257], dt.float32)
            nc.vector.tensor_scalar(ll[:], tt_[:], alo[:], None, Alu.mult)
            er = sp.tile([1, 257], dt.float32)
            nc.vector.tensor_tensor(er[:], e0[:], ll[:], Alu.add)
            ss = sp.tile([1, 257], dt.float32)
            nc.vector.tensor_tensor(ss[:], pr[:], cpart[:], Alu.add)
            bv = sp.tile([1, 257], dt.float32)
            nc.vector.tensor_tensor(bv[:], ss[:], pr[:], Alu.subtract)
            t4 = sp.tile([1, 257], dt.float32)
            nc.vector.tensor_tensor(t4[:], ss[:], bv[:], Alu.subtract)
            e2b = sp.tile([1, 257], dt.float32)
            nc.vector.tensor_tensor(e2b[:], pr[:], t4[:], Alu.subtract)
            e2c = sp.tile([1, 257], dt.float32)
            nc.vector.tensor_tensor(e2c[:], cpart[:], bv[:], Alu.subtract)
            e2 = sp.tile([1, 257], dt.float32)
            nc.vector.tensor_tensor(e2[:], e2b[:], e2c[:], Alu.add)
            corr = sp.tile([1, 257], dt.float32)
            nc.vector.tensor_tensor(corr[:], e2[:], er[:], Alu.add)
            edges = sp.tile([1, 257], dt.float32)
            nc.vector.tensor_tensor(edges[:], ss[:], corr[:], Alu.add)
            centers = sp.tile([1, 256], dt.float32)
            nc.vector.tensor_tensor(centers[:], edges[:, 0:256],
                                    edges[:, 1:257], Alu.add)
            nc.vector.tensor_scalar(centers[:], centers[:], 0.5, None,
                                    Alu.mult)

            # ---- Phase C: quantize -> ramp planes -> PE outer products ----
            NBANK = 4
            pt = [pp.tile([128, 128], dt.float32, name=f"pt{k}")
                  for k in range(NBANK)]
            ztp = pp.tile([16, 128], dt.float32, name="ztp")
            ysb = sp.tile([P, 128], dt.float32)
            zi32 = sp.tile([16, NBANK, 128], dt.int32)

            # engine split for the 32 ramp ops per chunk
            def ramp(eng, plane, src, i):
                if eng == "act":
                    nc.scalar.activation(plane, src,
                                         Act.Identity if i == 0 else Act.Relu,
                                         bias=biasr[:, i:i + 1], scale=1.0)
                elif i == 0:
                    if eng == "pool":
                        nc.gpsimd.tensor_scalar(plane, src, 1.0, None, Alu.add)
                    else:
                        nc.vector.tensor_scalar(plane, src, 1.0, None, Alu.add)
                else:
                    b = float(1 - i)
                    if eng == "pool":
                        nc.gpsimd.tensor_scalar(plane, src, b, 0.0,
                                                Alu.add, Alu.max)
                    else:
                        nc.vector.tensor_scalar(plane, src, b, 0.0,
                                                Alu.add, Alu.max)

            # (engine, side, plane) schedule: DVE ramps ~200ns, ACT ~614ns.
            # DVE takes 24 planes (all 16 fine + 8 coarse), ACT the other 8.
            SCHED = [("f", i) for i in range(16)] + [("c", i) for i in range(16)]
            ENG = (["dve"] * 23 + ["act"] * 9)

            with (
                tc.tile_pool(name="q", bufs=1 if dbg else 2) as qp,
                tc.tile_pool(name="planes", bufs=1 if dbg else 2) as plp,
            ):
                for ch in range(NPCH):
                    sl = slice(ch * PCH, (ch + 1) * PCH)
                    qc = qp.tile([P, PCH], dt.int16, tag="q")
                    nc.vector.tensor_scalar(qc[:], gray[:, sl], nadj[:],
                                            s256, Alu.add, Alu.mult)
                    cic = qp.tile([P, PCH], dt.int16, tag="ci")
                    nc.vector.tensor_scalar(cic[:], qc[:], 4, 15,
                                            Alu.logical_shift_right,
                                            Alu.bitwise_and)
                    fic = qp.tile([P, PCH], dt.int16, tag="fi")
                    nc.vector.tensor_scalar(fic[:], qc[:], 15, None,
                                            Alu.bitwise_and)
                    # fine side: weights-interleaved [block, plane, v] so each
                    # matmul's weights AP is one contiguous 128-element run
                    wfi = plp.tile([P, PCH // G, 16, G], dt.bfloat16, tag="rf")
                    rcp = plp.tile([P, 16, PCH], dt.bfloat16, tag="rc")
                    ffv = fic[:].rearrange("p (b v) -> p b v", v=G)
                    for k, (side, i) in enumerate(SCHED):
                        if side == "f":
                            ramp(ENG[k], wfi[:, :, i, :], ffv, i)
                        else:
                            ramp(ENG[k], rcp[:, i, :], cic[:], i)
                    bank = pt[ch % NBANK]
                    first, last = ch < NBANK, ch >= NPCH - NBANK
                    for v in range(0, PCH, G):
                        lw = wfi[:, v // G].rearrange("p i v -> p (i v)")
                        nc.tensor.matmul(bank[:], lhsT=lw,
                                         rhs=rcp[:, :, v:v + G],
                                         start=(first and v == 0),
                                         stop=(last and v == PCH - G))
                    if dbg and ch == NPCH - 1:
                        pf = sp.tile([1, 16, PCH], dt.float32, name="pfd")
                        nc.vector.tensor_copy(
                            pf[:], wfi[0:1].rearrange("o b i v -> o i (b v)"))
                        nc.sync.dma_start(
                            dbg_d[:, 16899:25091].rearrange(
                                "o (i v) -> o i v", i=16), pf[:])
                        pc = sp.tile([1, 16, PCH], dt.float32, name="pcd")
                        nc.vector.tensor_copy(pc[:], rcp[0:1])
                        nc.sync.dma_start(
                            dbg_d[:, 25091:33283].rearrange(
                                "o (i v) -> o i v", i=16), pc[:])
                        cfd = sp.tile([1, PCH], dt.float32, name="cfd")
                        nc.vector.tensor_copy(cfd[:], cic[0:1])
                        nc.sync.dma_start(dbg_d[:, 33283:33795], cfd[:])
                        ffd = sp.tile([1, PCH], dt.float32, name="ffd")
                        nc.vector.tensor_copy(ffd[:], fic[0:1])
                        nc.sync.dma_start(dbg_d[:, 33795:34307], ffd[:])
                    # bank (ch - (NPCH-NBANK)) is complete; extract diagonals
                    if ch >= NPCH - NBANK:
                        k = ch - (NPCH - NBANK)
                        nc.vector.tensor_tensor(ysb[:], pt[k][:], dmask[:],
                                                Alu.mult)
                        nc.tensor.matmul(ztp[:], lhsT=wsel[:], rhs=ysb[:],
                                         start=True, stop=True)
                        nc.vector.tensor_copy(zi32[:, k, :], ztp[:])

            if dbg:
                ptf = sp.tile([P, 128], dt.float32, name="ptf")
                nc.vector.tensor_copy(ptf[:], pt[0][:])
                nc.sync.dma_start(
                    dbg_d[:, 515:16899].rearrange(
                        "o (p n) -> (o p) n", p=P), ptf[:])
            # ---- Phase D: v+bank reduce (int32), double 2nd-difference ----
            # zi32[b, k, 8a+v] -> R[b, a] = sum over k, v
            zv = sp.tile([16, NBANK, 16], dt.int32)
            rmat = sp.tile([16, 16], dt.int32)
            with nc.allow_low_precision(reason="int32 adds are exact here"):
                nc.vector.tensor_reduce(
                    zv[:], zi32[:].rearrange("p k (a v) -> p k a v", v=G),
                    axis=Ax.X, op=Alu.add)
                nc.vector.tensor_reduce(
                    rmat[:], zv[:].rearrange("p k a -> p a k"),
                    axis=Ax.X, op=Alu.add)
            # Delta^2 over coarse (free axis): H1[:, c] = R[:,c] -2R[:,c+1] +R[:,c+2]
            h1 = sp.tile([16, 16], dt.int32)
            tsum = sp.tile([16, 14], dt.int32)
            nc.vector.tensor_tensor(tsum[:], rmat[:, 0:14], rmat[:, 2:16],
                                    Alu.add)
            nc.vector.scalar_tensor_tensor(h1[:, 0:14], rmat[:, 1:15], -2.0,
                                           tsum[:], Alu.mult, Alu.add)
            nc.vector.scalar_tensor_tensor(h1[:, 14:15], rmat[:, 15:16], -2.0,
                                           rmat[:, 14:15], Alu.mult, Alu.add)
            nc.vector.tensor_copy(h1[:, 15:16], rmat[:, 15:16])
            h1f = sp.tile([16, 16], dt.float32)
            nc.vector.tensor_copy(h1f[:], h1[:])
            # Delta^2 over fine (partition axis) via tiny constant matmul
            hps = pp.tile([16, 16], dt.float32, name="hps")
            nc.tensor.matmul(hps[:], lhsT=dmat[:], rhs=h1f[:],
                             start=True, stop=True)
            hist_sb = sp.tile([16, 16], dt.float32)
            nc.vector.tensor_copy(hist_sb[:], hps[:])
            h_in = dp.tile([1, 256], dt.float32)
            h_out = dp.tile([1, 256], dt.float32)
            nc.sync.dma_start(
                h_in[:].rearrange("o (f c) -> (o f) c", f=16), hist_sb[:])
            nc.gpsimd.collective_compute("AllReduce", Alu.add,
                                         replica_groups=groups,
                                         ins=[h_in.opt()],
                                         outs=[h_out.opt()])
            # h_out is f-major (16f + c); read back in bin order b = 16c + f
            hsb = sp.tile([1, 256], dt.float32)
            nc.sync.dma_start(
                hsb[:], h_out[:].rearrange("o (f c) -> o c f", f=16))

            # ---- Otsu on partition 0 ----
            zz = sp.tile([1, 256], dt.float32)
            nc.gpsimd.memset(zz[:], 0.0)
            w1 = sp.tile([1, 256], dt.float32)
            nc.vector.tensor_tensor_scan(w1[:], hsb[:], zz[:], 0.0,
                                         Alu.add, Alu.add)
            w2 = sp.tile([1, 256], dt.float32)
            nc.vector.tensor_tensor_scan(w2[:, ::-1], hsb[:, ::-1], zz[:],
                                         0.0, Alu.add, Alu.add)
            hc = sp.tile([1, 256], dt.float32)
            nc.vector.tensor_tensor(hc[:], hsb[:], centers[:], Alu.mult)
            s1 = sp.tile([1, 256], dt.float32)
            nc.vector.tensor_tensor_scan(s1[:], hc[:], zz[:], 0.0,
                                         Alu.add, Alu.add)
            s2 = sp.tile([1, 256], dt.float32)
            nc.vector.tensor_tensor_scan(s2[:, ::-1], hc[:, ::-1], zz[:],
                                         0.0, Alu.add, Alu.add)
            w1m = sp.tile([1, 256], dt.float32)
            nc.vector.tensor_scalar(w1m[:], w1[:], 1.0, None, Alu.max)
            w2m = sp.tile([1, 256], dt.float32)
            nc.vector.tensor_scalar(w2m[:], w2[:], 1.0, None, Alu.max)
            r1 = sp.tile([1, 256], dt.float32)
            nc.vector.reciprocal(r1[:], w1m[:])
            r2 = sp.tile([1, 256], dt.float32)
            nc.vector.reciprocal(r2[:], w2m[:])
            m1 = sp.tile([1, 256], dt.float32)
            nc.vector.tensor_tensor(m1[:], s1[:], r1[:], Alu.mult)
            m2 = sp.tile([1, 256], dt.float32)
            nc.vector.tensor_tensor(m2[:], s2[:], r2[:], Alu.mult)
            dmv = sp.tile([1, 255], dt.float32)
            nc.vector.tensor_tensor(dmv[:], m1[:, 0:255], m2[:, 1:256],
                                    Alu.subtract)
            d2 = sp.tile([1, 255], dt.float32)
            nc.vector.tensor_tensor(d2[:], dmv[:], dmv[:], Alu.mult)
            ww = sp.tile([1, 255], dt.float32)
            nc.vector.tensor_tensor(ww[:], w1[:, 0:255], w2[:, 1:256],
                                    Alu.mult)
            var = sp.tile([1, 255], dt.float32)
            nc.vector.tensor_tensor(var[:], ww[:], d2[:], Alu.mult)
            mx8 = sp.tile([1, 8], dt.float32)
            nc.vector.max(mx8[:], var[:])
            idx8 = sp.tile([1, 8], dt.uint32)
            nc.vector.max_index(idx8[:], mx8[:], var[:])
            idxf = sp.tile([1, 1], dt.float32)
            nc.vector.tensor_copy(idxf[:], idx8[:, 0:1])
            eqm = sp.tile([1, 256], dt.float32)
            nc.vector.tensor_scalar(eqm[:], iof[:, 0:256], idxf[:], None,
                                    Alu.is_equal)
            csel = sp.tile([1, 256], dt.float32)
            nc.vector.tensor_tensor(csel[:], eqm[:], centers[:], Alu.mult)
            thr11 = sp.tile([1, 1], dt.float32)
            nc.vector.tensor_reduce(thr11[:], csel[:], axis=Ax.X, op=Alu.add)
            thrb = sp.tile([P, 1], dt.float32)
            nc.gpsimd.partition_broadcast(thrb[:], thr11[:], channels=P)
            if dbg:
                nc.sync.dma_start(dbg_d[:, 0:2], mmg[:])
                nc.sync.dma_start(dbg_d[:, 2:258], hsb[:])
                nc.sync.dma_start(dbg_d[:, 258:259], thr11[:])
                rmf = sp.tile([16, 16], dt.float32)
                nc.vector.tensor_copy(rmf[:], rmat[:])
                nc.sync.dma_start(
                    dbg_d[:, 259:515].rearrange("o (b a) -> (o b) a", b=16),
                    rmf[:])
                zif = sp.tile([16, NBANK, 128], dt.float32)
                nc.vector.tensor_copy(zif[:], zi32[:])
                nc.sync.dma_start(
                    dbg_d[:, 34307:42499].rearrange(
                        "o (b k n) -> (o b) k n", b=16, k=NBANK),
                    zif[:])

            # ---- Phase E: binarize + replicate + store ----
            with tc.tile_pool(name="outp", bufs=3) as op_:
                for ch in range(NCH):
                    ot = op_.tile([P, CIN], dt.float32)
                    ov3 = ot[:].rearrange("p (v c) -> p v c", c=3)
                    gsb = gray[:, ch * CPIX:(ch + 1) * CPIX].unsqueeze(
                        2).to_broadcast((P, CPIX, 3))
                    nc.vector.tensor_scalar(ov3, gsb, thrb[:], None, Alu.is_gt)
                    nc.sync.dma_start(out_d[:, ch * CIN:(ch + 1) * CIN], ot[:])

    nc.compile()
    return nc


def get_nc():
    if "nc" not in _NC_CACHE:
        _NC_CACHE["nc"] = _build_nc()
    return _NC_CACHE["nc"]


def _shard(x):
    x = np.ascontiguousarray(x, dtype=np.float32)
    return [x[c * BPC:(c + 1) * BPC].reshape(P, FIN) for c in range(NCORES)]


def kernel(inputs):
    from concourse.bass_utils import run_bass_kernel_spmd

    nc = get_nc()
    in_maps = [{"x": s} for s in _shard(inputs)]
    res = run_bass_kernel_spmd(nc, in_maps, core_ids=list(range(NCORES)))
    out = np.concatenate(
        [res.results[c]["out"].reshape(BPC, H, WD, C) for c in range(NCORES)],
        axis=0)
    return out
